# revision 1
# baseline (speedup 1.0000x reference)
"""Linformer text encoder on 8 TRN2 NeuronCores.

Sharding: pure data-parallel over batch (32 seqs -> 4 per core), weights
replicated, no collectives. Host does the embedding gather (cheaper to stage
32MB/core of gathered rows than 205MB/core of table) and folds LN gamma/beta
into the following projection weights (exact math). Device does everything
else in bf16 matmuls with f32 accumulation.

Self-contained: hardcodes all shapes from the problem spec.
"""

import sys

sys.path.insert(0, "/opt/trn_rl_repo")

from contextlib import ExitStack
from dataclasses import dataclass

import ml_dtypes
import numpy as np

import concourse.bass as bass
import concourse.tile as tile
from concourse import bacc, mybir
from concourse.bass_utils import run_bass_kernel_spmd
from concourse.masks import make_identity

F32 = mybir.dt.float32
BF16 = mybir.dt.bfloat16
AF = mybir.ActivationFunctionType
ALU = mybir.AluOpType
AX = mybir.AxisListType

EPS = 1e-5


@dataclass(frozen=True)
class Dims:
    B_loc: int = 4      # sequences per core
    N: int = 2048       # tokens per sequence
    D: int = 1024
    H: int = 16
    DH: int = 64
    K: int = 64
    FF: int = 4096
    L: int = 4

    @property
    def R(self):
        return self.B_loc * self.N

    @property
    def n_blk(self):        # 128-token blocks per sequence
        return self.N // 128

    @property
    def n_chunk(self):      # 512-token chunks per sequence
        return self.N // 512


def _ln_stats(nc, pool, xt, d, eps_t, out_rstd=None, out_nmr=None):
    """Token-major LN stats for xt [128, d] f32.
    Returns (rstd [P,1] f32, neg_mean_rstd [P,1] f32).
    Uses exp(-0.5*ln(var+eps)) so the whole kernel stays inside the
    natural_log_exp activation-table set (no LUT reloads)."""
    P = xt.shape[0]
    ngrp = d // 512
    bns = pool.tile([P, ngrp, 6], F32, tag="bns")
    for g in range(ngrp):
        nc.vector.bn_stats(bns[:, g, :], xt[:, g * 512:(g + 1) * 512])
    mv = pool.tile([P, 2], F32, tag="mv")
    nc.vector.bn_aggr(mv[:], bns[:])
    lnv = pool.tile([P, 1], F32, tag="lnv")
    nc.scalar.activation(lnv[:], mv[:, 1:2], AF.Ln, bias=eps_t[:P, :])
    rstd = out_rstd if out_rstd is not None else pool.tile([P, 1], F32, tag="rstd")
    nc.scalar.activation(rstd[:], lnv[:], AF.Exp, scale=-0.5)
    nmr = out_nmr if out_nmr is not None else pool.tile([P, 1], F32, tag="nmr")
    # nmr = (mean * -1) * rstd
    nc.vector.scalar_tensor_tensor(nmr[:], mv[:, 0:1], -1.0, rstd[:], ALU.mult, ALU.mult)
    return rstd, nmr


def build(dims: Dims, n_cores: int, biases):
    """Emit the full per-core program. `biases` is a dict of host numpy
    vectors (cq, kvc, kvtc, bu, bo, bz per layer) or None entries when zero."""
    d = dims
    nc = bacc.Bacc("TRN2", target_bir_lowering=False, debug=False,
                   num_devices=n_cores, enable_asserts=False)

    x0 = nc.dram_tensor("x0", [d.R, d.D], F32, kind="ExternalInput")
    wq_d = [nc.dram_tensor(f"wq{l}", [d.D, d.D], BF16, kind="ExternalInput") for l in range(d.L)]
    wk_d = [nc.dram_tensor(f"wk{l}", [d.D, d.DH], BF16, kind="ExternalInput") for l in range(d.L)]
    pk_d = [nc.dram_tensor(f"pk{l}", [d.N, d.K], BF16, kind="ExternalInput") for l in range(d.L)]
    wo_d = [nc.dram_tensor(f"wo{l}", [d.D, d.D], BF16, kind="ExternalInput") for l in range(d.L)]
    w1_d = [nc.dram_tensor(f"w1{l}", [d.D, d.FF], BF16, kind="ExternalInput") for l in range(d.L)]
    w2_d = [nc.dram_tensor(f"w2{l}", [d.FF, d.D], BF16, kind="ExternalInput") for l in range(d.L)]
    lnfg = nc.dram_tensor("lnfg", [1, d.D], F32, kind="ExternalInput")
    lnfb = nc.dram_tensor("lnfb", [1, d.D], F32, kind="ExternalInput")
    out = nc.dram_tensor("out", [d.B_loc, d.D], F32, kind="ExternalOutput")

    bias_d = {}
    for l in range(d.L):
        for nm in ("cq", "kvc", "kvtc", "bu", "bo", "bz"):
            if biases and biases.get((nm, l)) is not None:
                arr = biases[(nm, l)]
                dt = F32 if nm in ("kvc", "kvtc") else BF16
                bias_d[(nm, l)] = nc.dram_tensor(
                    f"{nm}{l}", list(arr.shape), dt, kind="ExternalInput")

    Xp = nc.dram_tensor("Xp", [d.R, d.D], F32)   # post-attention residual
    Xr = nc.dram_tensor("Xr", [d.R, d.D], F32)   # post-FF residual

    nD = d.D // 128       # 8 feature chunks
    nF = d.FF // 128      # 32 ff chunks

    with ExitStack() as ctx:
        tc = ctx.enter_context(tile.TileContext(nc))
        const = ctx.enter_context(tc.tile_pool(name="const", bufs=1))
        small = ctx.enter_context(tc.tile_pool(name="small", bufs=4))
        pmm = ctx.enter_context(tc.tile_pool(name="pmm", bufs=4, space="PSUM"))
        ptr = ctx.enter_context(tc.tile_pool(name="ptr", bufs=2, space="PSUM"))
        pkv = ctx.enter_context(tc.tile_pool(name="pkv", bufs=1, space="PSUM"))

        idt = const.tile([128, 128], BF16)
        make_identity(nc, idt[:])
        ones_bf = const.tile([1, 512], BF16)
        nc.vector.memset(ones_bf[:], 1.0)
        ones_f32 = const.tile([128, 1], F32)
        nc.vector.memset(ones_f32[:], 1.0)
        eps_t = const.tile([128, 1], F32)
        nc.vector.memset(eps_t[:], EPS)
        ilo = const.tile([64, 128], BF16)
        nc.vector.memset(ilo[:], 0.0)
        make_identity(nc, ilo[:, 0:64])
        ihi = const.tile([64, 128], BF16)
        nc.vector.memset(ihi[:], 0.0)
        make_identity(nc, ihi[:, 64:128])

        def load_bias_rows(pool, l, names):
            out = {}
            for nm in names:
                if (nm, l) in bias_d:
                    dram = bias_d[(nm, l)]
                    t = pool.tile([1, dram.shape[0]], BF16, tag=f"b{nm}",
                                  name=f"b{nm}{l}")
                    nc.sync.dma_start(t[:], dram.ap()[None, :])
                    out[(nm, l)] = t
            return out

        def ln_block(src_ap, rows, pool_x, pool_h):
            """Load [128, D] f32 rows from DRAM, layernorm -> bf16 h."""
            xt = pool_x.tile([128, d.D], F32, tag="xt")
            nc.sync.dma_start(xt[:], src_ap[rows * 128:(rows + 1) * 128, :])
            rstd, nmr = _ln_stats(nc, small, xt, d.D, eps_t)
            h = pool_h.tile([128, d.D], BF16, tag="h")
            nc.scalar.activation(h[:], xt[:], AF.Identity, bias=nmr[:], scale=rstd[:])
            return xt, h

        def transpose_into(h_ap, dst_tile, dst_chunk0, tcol):
            """Transpose h_ap [128, D or chunk...]: for each 128-col chunk c,
            write h^T chunk into dst_tile[:, dst_chunk0+c, tcol*128:+128]."""
            nch = h_ap.shape[1] // 128
            for c0 in range(0, nch, 4):
                cn = min(4, nch - c0)
                pt = ptr.tile([128, 512], BF16, tag="pt")
                for c in range(cn):
                    nc.tensor.transpose(
                        pt[:, c * 128:(c + 1) * 128],
                        h_ap[:, (c0 + c) * 128:(c0 + c + 1) * 128], idt[:])
                nc.vector.tensor_copy(
                    dst_tile[:, dst_chunk0 + c0:dst_chunk0 + c0 + cn,
                             tcol * 128:(tcol + 1) * 128],
                    pt[:, :cn * 128].rearrange("p (a f) -> p a f", a=cn))

        for l in range(d.L):
            src = x0 if l == 0 else Xr
            stp = ctx.enter_context(tc.tile_pool(name=f"st{l}", bufs=1))
            s2 = stp.tile([128, d.R // 128, 2], F32, name=f"s2_{l}")

            # ---------------- attention: pass A + pass B1, per sequence ----
            with tc.tile_pool(name=f"wat{l}", bufs=1) as wat, \
                 tc.tile_pool(name=f"pha{l}", bufs=2) as htp, \
                 tc.tile_pool(name=f"wka{l}", bufs=2) as work, \
                 tc.tile_pool(name=f"xa{l}", bufs=3) as xin, \
                 tc.tile_pool(name=f"ha{l}", bufs=2) as hbuf, \
                 tc.tile_pool(name=f"oa{l}", bufs=3) as outp:
                wqS = wat.tile([128, nD, d.D], BF16, tag="wq")
                nc.sync.dma_start(wqS[:], wq_d[l].ap().rearrange("(a p) n -> p a n", p=128))
                wkS = wat.tile([128, nD, d.DH], BF16, tag="wk")
                nc.sync.dma_start(wkS[:], wk_d[l].ap().rearrange("(a p) n -> p a n", p=128))
                pkS = wat.tile([128, d.n_blk, d.K], BF16, tag="pk")
                nc.sync.dma_start(pkS[:], pk_d[l].ap().rearrange("(a p) k -> p a k", p=128))
                woS = wat.tile([128, nD, d.D], BF16, tag="wo")
                nc.sync.dma_start(woS[:], wo_d[l].ap().rearrange("(a p) n -> p a n", p=128))
                bias_sb = load_bias_rows(wat, l, ("cq", "bo"))

                for b in range(d.B_loc):
                    # ---- pass A: LN, h^T cache, hk, kv/kvT accumulation ----
                    hT = htp.tile([128, nD, d.N], BF16, tag="hT")
                    kvT_ps = pkv.tile([64, 64], F32, tag="kvT")
                    kv_ps = pkv.tile([64, 64], F32, tag="kv")
                    for t in range(d.n_blk):
                        r = b * d.n_blk + t
                        xt, h = ln_block(src.ap(), r, xin, hbuf)
                        transpose_into(h[:], hT, 0, t)
                        hk_ps = pmm.tile([128, d.DH], F32, tag="mm")
                        for dc in range(nD):
                            nc.tensor.matmul(
                                hk_ps[:], hT[:, dc, t * 128:(t + 1) * 128],
                                wkS[:, dc, :], start=(dc == 0), stop=(dc == nD - 1))
                        hk = work.tile([128, d.DH], BF16, tag="hk")
                        nc.vector.tensor_copy(hk[:], hk_ps[:])
                        nc.tensor.matmul(kvT_ps[:], hk[:], pkS[:, t, :],
                                         start=(t == 0), stop=(t == d.n_blk - 1))
                        nc.tensor.matmul(kv_ps[:], pkS[:, t, :], hk[:],
                                         start=(t == 0), stop=(t == d.n_blk - 1))
                    kvT = work.tile([64, 64], BF16, tag="kvT")
                    kv = work.tile([64, 64], BF16, tag="kv")
                    if ("kvtc", l) in bias_d:
                        kvc_t = work.tile([64, 64], F32, tag="kvtf")
                        nc.sync.dma_start(kvc_t[:], bias_d[("kvtc", l)].ap())
                        nc.vector.tensor_add(kvT[:], kvT_ps[:], kvc_t[:])
                        kvc2 = work.tile([64, 64], F32, tag="kvcf")
                        nc.sync.dma_start(kvc2[:], bias_d[("kvc", l)].ap())
                        nc.vector.tensor_add(kv[:], kv_ps[:], kvc2[:])
                    else:
                        nc.vector.tensor_copy(kvT[:], kvT_ps[:])
                        nc.vector.tensor_copy(kv[:], kv_ps[:])
                    # block-diagonal duplicates: bdT = blockdiag(kvT,kvT),
                    # bdv = blockdiag(kv,kv) -- lets head-pair matmuls run with
                    # 128-partition operands all based at partition 0
                    bdT_ps = pmm.tile([128, 128], F32, tag="mm", name="bdTps")
                    nc.tensor.matmul(bdT_ps[:, 0:64], ilo[:], kvT[:])
                    nc.tensor.matmul(bdT_ps[:, 64:128], ihi[:], kvT[:])
                    bdv_ps = pmm.tile([128, 128], F32, tag="mm", name="bdvps")
                    nc.tensor.matmul(bdv_ps[:, 0:64], ilo[:], kv[:])
                    nc.tensor.matmul(bdv_ps[:, 64:128], ihi[:], kv[:])
                    bdT = work.tile([128, 128], BF16, tag="bdT")
                    nc.vector.tensor_copy(bdT[:], bdT_ps[:])
                    bdv = work.tile([128, 128], BF16, tag="bdv")
                    nc.vector.tensor_copy(bdv[:], bdv_ps[:])

                    # ---- pass B1: q, dots, softmax, o, Wo, residual -------
                    for c4 in range(d.n_chunk):
                        tok0 = c4 * 512
                        qT = work.tile([128, nD, 512], BF16, tag="qT")
                        for ncol in range(nD):
                            q_ps = pmm.tile([128, 512], F32, tag="mm")
                            for dc in range(nD):
                                nc.tensor.matmul(
                                    q_ps[:], wqS[:, dc, ncol * 128:(ncol + 1) * 128],
                                    hT[:, dc, tok0:tok0 + 512],
                                    start=(dc == 0),
                                    stop=(dc == nD - 1 and ("cq", l) not in bias_sb))
                            if ("cq", l) in bias_sb:
                                nc.tensor.matmul(
                                    q_ps[:], bias_sb[("cq", l)][:, ncol * 128:(ncol + 1) * 128],
                                    ones_bf[:], start=False, stop=True)
                            nc.vector.tensor_copy(qT[:, ncol, :], q_ps[:])

                        for tb in range(4):
                            t = c4 * 4 + tb
                            r = b * d.n_blk + t
                            # dots: two psum tiles cover 16 heads
                            dots_ps = [pmm.tile([128, 512], F32, tag="mm", name=f"dots{j}") for j in range(2)]
                            for c in range(nD):
                                nc.tensor.matmul(
                                    dots_ps[c // 4][:, (c % 4) * 128:(c % 4 + 1) * 128],
                                    qT[:, c, tb * 128:(tb + 1) * 128],
                                    bdT[:])
                            expt = work.tile([128, d.H * d.K], F32, tag="expt")
                            for j in range(2):
                                nc.scalar.activation(expt[:, j * 512:(j + 1) * 512],
                                                     dots_ps[j][:], AF.Exp,
                                                     scale=float(d.DH) ** -0.5)
                            se = small.tile([128, d.H], F32, tag="se")
                            nc.vector.reduce_sum(
                                se[:], expt[:].rearrange("p (h k) -> p h k", h=d.H),
                                axis=AX.X)
                            rse = small.tile([128, d.H], F32, tag="rse")
                            nc.vector.reciprocal(rse[:], se[:])
                            attn = work.tile([128, d.H * d.K], BF16, tag="attn")
                            for h_i in range(d.H):
                                nc.vector.tensor_scalar_mul(
                                    attn[:, h_i * 64:(h_i + 1) * 64],
                                    expt[:, h_i * 64:(h_i + 1) * 64],
                                    rse[:, h_i:h_i + 1])
                            attnT = work.tile([128, nD, 128], BF16, tag="attnT")
                            transpose_into(attn[:], attnT, 0, 0)
                            oT_ps = [pmm.tile([128, 512], F32, tag="mm", name=f"oTps{j}") for j in range(2)]
                            for c in range(nD):
                                nc.tensor.matmul(
                                    oT_ps[c // 4][:, (c % 4) * 128:(c % 4 + 1) * 128],
                                    bdv[:],
                                    attnT[:, c, :])
                            oT = work.tile([128, nD, 128], BF16, tag="oT")
                            for j in range(2):
                                nc.vector.tensor_copy(
                                    oT[:, j * 4:(j + 1) * 4, :],
                                    oT_ps[j][:].rearrange("p (a f) -> p a f", a=4))
                            # y = oT^T @ Wo (+bo) ; X' = X + y
                            xb = xin.tile([128, d.D], F32, tag="xres")
                            nc.sync.dma_start(xb[:], src.ap()[r * 128:(r + 1) * 128, :])
                            xp = outp.tile([128, d.D], F32, tag="xp")
                            for ncol in range(2):
                                y_ps = pmm.tile([128, 512], F32, tag="mm")
                                for dc in range(nD):
                                    nc.tensor.matmul(
                                        y_ps[:], oT[:, dc, :],
                                        woS[:, dc, ncol * 512:(ncol + 1) * 512],
                                        start=(dc == 0),
                                        stop=(dc == nD - 1 and ("bo", l) not in bias_sb))
                                if ("bo", l) in bias_sb:
                                    nc.tensor.matmul(
                                        y_ps[:], ones_bf[:, 0:128],
                                        bias_sb[("bo", l)][:, ncol * 512:(ncol + 1) * 512],
                                        start=False, stop=True)
                                nc.vector.scalar_tensor_tensor(
                                    xp[:, ncol * 512:(ncol + 1) * 512], y_ps[:], 1.0,
                                    xb[:, ncol * 512:(ncol + 1) * 512], ALU.mult, ALU.add)
                            _ln_stats(nc, small, xp, d.D, eps_t,
                                      out_rstd=s2[:, r, 0:1], out_nmr=s2[:, r, 1:2])
                            nc.sync.dma_start(Xp.ap()[r * 128:(r + 1) * 128, :], xp[:])

            # ---------------- FF: pass B2, per 512-token chunk -------------
            with tc.tile_pool(name=f"wff{l}", bufs=1) as wff, \
                 tc.tile_pool(name=f"phf{l}", bufs=1) as htp, \
                 tc.tile_pool(name=f"xf{l}", bufs=2) as xin, \
                 tc.tile_pool(name=f"hf{l}", bufs=1) as hbuf, \
                 tc.tile_pool(name=f"of{l}", bufs=1) as outp:
                w1S = wff.tile([128, nD, d.FF], BF16, tag="w1")
                nc.sync.dma_start(w1S[:], w1_d[l].ap().rearrange("(a p) n -> p a n", p=128))
                w2S = wff.tile([128, nF, d.D], BF16, tag="w2")
                nc.sync.dma_start(w2S[:], w2_d[l].ap().rearrange("(a p) n -> p a n", p=128))
                bias_sb = load_bias_rows(wff, l, ("bu", "bz"))

                for cg in range(d.R // 512):
                    h2T = htp.tile([128, nD, 512], BF16, tag="h2T")
                    for tb in range(4):
                        r = cg * 4 + tb
                        xt = xin.tile([128, d.D], F32, tag="xt")
                        nc.sync.dma_start(xt[:], Xp.ap()[r * 128:(r + 1) * 128, :])
                        h2 = hbuf.tile([128, d.D], BF16, tag="h")
                        nc.scalar.activation(h2[:], xt[:], AF.Identity,
                                             bias=s2[:, r, 1:2], scale=s2[:, r, 0:1])
                        transpose_into(h2[:], h2T, 0, tb)
                    uT = htp.tile([128, nF, 512], BF16, tag="uT")
                    for fc in range(nF):
                        u_ps = pmm.tile([128, 512], F32, tag="mm")
                        for dc in range(nD):
                            nc.tensor.matmul(
                                u_ps[:], w1S[:, dc, fc * 128:(fc + 1) * 128],
                                h2T[:, dc, :], start=(dc == 0),
                                stop=(dc == nD - 1 and ("bu", l) not in bias_sb))
                        if ("bu", l) in bias_sb:
                            nc.tensor.matmul(
                                u_ps[:], bias_sb[("bu", l)][:, fc * 128:(fc + 1) * 128],
                                ones_bf[:], start=False, stop=True)
                        nc.scalar.activation(uT[:, fc, :], u_ps[:], AF.Gelu)
                    for tb in range(4):
                        r = cg * 4 + tb
                        xres = xin.tile([128, d.D], F32, tag="xres", bufs=1)
                        nc.sync.dma_start(xres[:], Xp.ap()[r * 128:(r + 1) * 128, :])
                        xo = outp.tile([128, d.D], F32, tag="xo")
                        for ncol in range(2):
                            z_ps = pmm.tile([128, 512], F32, tag="mm")
                            for fc in range(nF):
                                nc.tensor.matmul(
                                    z_ps[:], uT[:, fc, tb * 128:(tb + 1) * 128],
                                    w2S[:, fc, ncol * 512:(ncol + 1) * 512],
                                    start=(fc == 0),
                                    stop=(fc == nF - 1 and ("bz", l) not in bias_sb))
                            if ("bz", l) in bias_sb:
                                nc.tensor.matmul(
                                    z_ps[:], ones_bf[:, 0:128],
                                    bias_sb[("bz", l)][:, ncol * 512:(ncol + 1) * 512],
                                    start=False, stop=True)
                            nc.vector.scalar_tensor_tensor(
                                xo[:, ncol * 512:(ncol + 1) * 512], z_ps[:], 1.0,
                                xres[:, ncol * 512:(ncol + 1) * 512],
                                ALU.mult, ALU.add)
                        nc.sync.dma_start(Xr.ap()[r * 128:(r + 1) * 128, :], xo[:])

        # ---------------- final: mean over tokens, layernorm ---------------
        fin = ctx.enter_context(tc.tile_pool(name="fin", bufs=2))
        gt = fin.tile([1, d.D], F32, tag="lnfg", bufs=1)
        nc.sync.dma_start(gt[:], lnfg.ap())
        bt = fin.tile([1, d.D], F32, tag="lnfb", bufs=1)
        nc.sync.dma_start(bt[:], lnfb.ap())
        for b in range(d.B_loc):
            e_ps = [pmm.tile([1, 512], F32, tag="mm", name=f"eps{j}") for j in range(2)]
            for t in range(d.n_blk):
                r = b * d.n_blk + t
                xb = fin.tile([128, d.D], F32, tag="xt")
                nc.sync.dma_start(xb[:], Xr.ap()[r * 128:(r + 1) * 128, :])
                for j in range(2):
                    nc.tensor.matmul(e_ps[j][:], ones_f32[:], xb[:, j * 512:(j + 1) * 512],
                                     start=(t == 0), stop=(t == d.n_blk - 1))
            emb = fin.tile([1, d.D], F32, tag="emb")
            for j in range(2):
                nc.scalar.mul(emb[:, j * 512:(j + 1) * 512], e_ps[j][:], 1.0 / d.N)
            rstd, nmr = _ln_stats(nc, small, emb, d.D, eps_t)
            nrm = fin.tile([1, d.D], F32, tag="nrm")
            nc.scalar.activation(nrm[:], emb[:], AF.Identity, bias=nmr[:], scale=rstd[:])
            ot = fin.tile([1, d.D], F32, tag="ot")
            nc.vector.tensor_mul(ot[:], nrm[:], gt[:])
            nc.vector.tensor_add(ot[:], ot[:], bt[:])
            nc.sync.dma_start(out.ap()[b:b + 1, :], ot[:])

    nc.compile()
    return nc


_CACHE = {}


def _to_bf16(a):
    return np.asarray(a, dtype=np.float32).astype(ml_dtypes.bfloat16)


def prepare_inputs(dims: Dims, n_cores, token_ids, token_emb, pos_emb, ln1_g, ln1_b,
                   Wq, Wk, Pk, Wo, bo, ln2_g, ln2_b, W1, b1, W2, b2, lnf_g, lnf_b):
    d = dims
    token_ids = np.asarray(token_ids)
    token_emb = np.asarray(token_emb, dtype=np.float32)
    pos_emb = np.asarray(pos_emb, dtype=np.float32)

    x_all = token_emb[token_ids[:, :d.N]] + pos_emb[None, :d.N, :]  # [B, N, D]
    B = token_ids.shape[0]
    assert B == n_cores * d.B_loc

    biases = {}
    shared = {}
    for l in range(d.L):
        g1 = np.asarray(ln1_g[l], np.float32)
        b1l = np.asarray(ln1_b[l], np.float32)
        g2 = np.asarray(ln2_g[l], np.float32)
        b2l = np.asarray(ln2_b[l], np.float32)
        Wql = np.asarray(Wq[l], np.float32)
        Wkl = np.asarray(Wk[l], np.float32)
        W1l = np.asarray(W1[l], np.float32)
        shared[f"wq{l}"] = _to_bf16(g1[:, None] * Wql)
        shared[f"wk{l}"] = _to_bf16(g1[:, None] * Wkl)
        shared[f"pk{l}"] = _to_bf16(np.asarray(Pk[l])[:d.N])
        shared[f"wo{l}"] = _to_bf16(Wo[l])
        shared[f"w1{l}"] = _to_bf16(g2[:, None] * W1l)
        shared[f"w2{l}"] = _to_bf16(W2[l])

        def nz(v):
            v = np.asarray(v, np.float32)
            return v if np.any(v != 0) else None

        cq = nz(b1l @ Wql)
        ck = b1l @ Wkl                                   # [DH]
        colsum = np.asarray(Pk[l], np.float32)[:d.N].sum(axis=0)   # [K]
        kvc = nz(np.outer(colsum, ck))                   # [K, DH]
        bul = nz(b2l @ W1l + np.asarray(b1[l], np.float32))
        bol = nz(bo[l])
        bzl = nz(b2[l])
        biases[("cq", l)] = _to_bf16(cq) if cq is not None else None
        biases[("kvc", l)] = kvc.astype(np.float32) if kvc is not None else None
        biases[("kvtc", l)] = kvc.T.astype(np.float32).copy() if kvc is not None else None
        biases[("bu", l)] = _to_bf16(bul) if bul is not None else None
        biases[("bo", l)] = _to_bf16(bol) if bol is not None else None
        biases[("bz", l)] = _to_bf16(bzl) if bzl is not None else None

    lnf_g_rep = np.asarray(lnf_g, np.float32).reshape(1, d.D).copy()
    lnf_b_rep = np.asarray(lnf_b, np.float32).reshape(1, d.D).copy()

    in_maps = []
    for c in range(n_cores):
        m = dict(shared)
        m["x0"] = np.ascontiguousarray(
            x_all[c * d.B_loc:(c + 1) * d.B_loc].reshape(d.R, d.D), dtype=np.float32)
        m["lnfg"] = lnf_g_rep
        m["lnfb"] = lnf_b_rep
        for key, v in biases.items():
            if v is not None:
                m[f"{key[0]}{key[1]}"] = v
        in_maps.append(m)
    return in_maps, biases


def run(dims: Dims, n_cores, inputs, trace=False, tmpdir=None):
    in_maps, biases = prepare_inputs(dims, n_cores, **inputs)
    ck = (dims, n_cores, tuple(sorted(k for k, v in biases.items() if v is not None)))
    if ck not in _CACHE:
        _CACHE[ck] = build(dims, n_cores, biases)
    nc = _CACHE[ck]
    res = run_bass_kernel_spmd(nc, in_maps, list(range(n_cores)), trace=trace,
                               tmpdir=tmpdir)
    outs = np.concatenate([res.results[i]["out"] for i in range(n_cores)], axis=0)
    return outs, res


def kernel(**inputs) -> np.ndarray:
    out, _ = run(Dims(), 8, inputs)
    return out.astype(np.float32)



# revision 11
# speedup vs baseline: 1.0840x; 1.0840x over previous
"""Linformer text encoder on 8 TRN2 NeuronCores.

Sharding: pure data-parallel over batch (32 seqs -> 4 per core), weights
replicated, no collectives. Host does the embedding gather (cheaper to stage
32MB/core of gathered rows than 205MB/core of table) and folds LN gamma/beta
into the following projection weights (exact math). Device does everything
else in bf16 matmuls with f32 accumulation.

Self-contained: hardcodes all shapes from the problem spec.
"""

import sys

sys.path.insert(0, "/opt/trn_rl_repo")

from contextlib import ExitStack
from dataclasses import dataclass

import ml_dtypes
import numpy as np

import concourse.bass as bass
import concourse.tile as tile
from concourse import bacc, mybir
from concourse.bass_utils import run_bass_kernel_spmd
from concourse.masks import make_identity

F32 = mybir.dt.float32
BF16 = mybir.dt.bfloat16
AF = mybir.ActivationFunctionType
ALU = mybir.AluOpType
AX = mybir.AxisListType

EPS = 1e-5


@dataclass(frozen=True)
class Dims:
    B_loc: int = 4      # sequences per core
    N: int = 2048       # tokens per sequence
    D: int = 1024
    H: int = 16
    DH: int = 64
    K: int = 64
    FF: int = 4096
    L: int = 4

    @property
    def R(self):
        return self.B_loc * self.N

    @property
    def n_blk(self):        # 128-token blocks per sequence
        return self.N // 128

    @property
    def n_chunk(self):      # 512-token chunks per sequence
        return self.N // 512


def _ln_stats(nc, pool, xt, d, eps_t, out_rstd=None, out_nmr=None):
    """Token-major LN stats for xt [128, d] f32.
    Returns (rstd [P,1] f32, neg_mean_rstd [P,1] f32).
    Uses exp(-0.5*ln(var+eps)) so the whole kernel stays inside the
    natural_log_exp activation-table set (no LUT reloads)."""
    P = xt.shape[0]
    ngrp = d // 512
    bns = pool.tile([P, ngrp, 6], F32, tag="bns")
    for g in range(ngrp):
        nc.vector.bn_stats(bns[:, g, :], xt[:, g * 512:(g + 1) * 512])
    mv = pool.tile([P, 2], F32, tag="mv")
    nc.vector.bn_aggr(mv[:], bns[:])
    lnv = pool.tile([P, 1], F32, tag="lnv")
    nc.scalar.activation(lnv[:], mv[:, 1:2], AF.Ln, bias=eps_t[:P, :])
    rstd = out_rstd if out_rstd is not None else pool.tile([P, 1], F32, tag="rstd")
    nc.scalar.activation(rstd[:], lnv[:], AF.Exp, scale=-0.5)
    nmr = out_nmr if out_nmr is not None else pool.tile([P, 1], F32, tag="nmr")
    # nmr = (mean * -1) * rstd
    nc.vector.scalar_tensor_tensor(nmr[:], mv[:, 0:1], -1.0, rstd[:], ALU.mult, ALU.mult)
    return rstd, nmr


def build(dims: Dims, n_cores: int, biases):
    """Emit the full per-core program. `biases` is a dict of host numpy
    vectors (cq, kvc, kvtc, bu, bo, bz per layer) or None entries when zero."""
    d = dims
    nc = bacc.Bacc("TRN2", target_bir_lowering=False, debug=False,
                   num_devices=n_cores, enable_asserts=False)

    x0 = nc.dram_tensor("x0", [d.R, d.D], F32, kind="ExternalInput")
    wq_d = [nc.dram_tensor(f"wq{l}", [d.D, d.D], BF16, kind="ExternalInput") for l in range(d.L)]
    wk_d = [nc.dram_tensor(f"wk{l}", [d.D, d.DH], BF16, kind="ExternalInput") for l in range(d.L)]
    pk_d = [nc.dram_tensor(f"pk{l}", [d.N, d.K], BF16, kind="ExternalInput") for l in range(d.L)]
    wo_d = [nc.dram_tensor(f"wo{l}", [d.D, d.D], BF16, kind="ExternalInput") for l in range(d.L)]
    w1_d = [nc.dram_tensor(f"w1{l}", [d.D, d.FF], BF16, kind="ExternalInput") for l in range(d.L)]
    w2_d = [nc.dram_tensor(f"w2{l}", [d.FF, d.D], BF16, kind="ExternalInput") for l in range(d.L)]
    lnfg = nc.dram_tensor("lnfg", [1, d.D], F32, kind="ExternalInput")
    lnfb = nc.dram_tensor("lnfb", [1, d.D], F32, kind="ExternalInput")
    out = nc.dram_tensor("out", [d.B_loc, d.D], F32, kind="ExternalOutput")

    bias_d = {}
    for l in range(d.L):
        for nm in ("cq", "kvc", "kvtc", "bu", "bo", "bz"):
            if biases and biases.get((nm, l)) is not None:
                arr = biases[(nm, l)]
                dt = F32 if nm in ("kvc", "kvtc") else BF16
                bias_d[(nm, l)] = nc.dram_tensor(
                    f"{nm}{l}", list(arr.shape), dt, kind="ExternalInput")

    Xp = nc.dram_tensor("Xp", [d.R, d.D], F32)   # post-attention residual
    Xr = nc.dram_tensor("Xr", [d.R, d.D], F32)   # post-FF residual

    nD = d.D // 128       # 8 feature chunks
    nF = d.FF // 128      # 32 ff chunks

    with ExitStack() as ctx:
        tc = ctx.enter_context(tile.TileContext(nc))
        const = ctx.enter_context(tc.tile_pool(name="const", bufs=1))
        small = ctx.enter_context(tc.tile_pool(name="small", bufs=4))
        mup = ctx.enter_context(tc.tile_pool(name="mup", bufs=1))
        mup = ctx.enter_context(tc.tile_pool(name="mup", bufs=1))
        pmm = ctx.enter_context(tc.tile_pool(name="pmm", bufs=4, space="PSUM"))
        ptr = ctx.enter_context(tc.tile_pool(name="ptr", bufs=2, space="PSUM"))
        pkv = ctx.enter_context(tc.tile_pool(name="pkv", bufs=1, space="PSUM"))

        mus = [mup.tile([128, 32], F32, tag=f"mu{b}", name=f"mu{b}")
               for b in range(4)]
        idt = const.tile([128, 128], BF16)
        make_identity(nc, idt[:])
        ones_bf = const.tile([1, 512], BF16)
        nc.vector.memset(ones_bf[:], 1.0)
        ones_f32 = const.tile([128, 1], F32)
        nc.vector.memset(ones_f32[:], 1.0)
        eps_t = const.tile([128, 1], F32)
        nc.vector.memset(eps_t[:], EPS)
        ilo = const.tile([64, 128], BF16)
        nc.vector.memset(ilo[:], 0.0)
        make_identity(nc, ilo[:, 0:64])
        ihi = const.tile([64, 128], BF16)
        nc.vector.memset(ihi[:], 0.0)
        make_identity(nc, ihi[:, 64:128])

        def load_bias_rows(pool, l, names):
            out = {}
            for nm in names:
                if (nm, l) in bias_d:
                    dram = bias_d[(nm, l)]
                    t = pool.tile([1, dram.shape[0]], BF16, tag=f"b{nm}",
                                  name=f"b{nm}{l}")
                    nc.sync.dma_start(t[:], dram.ap()[None, :])
                    out[(nm, l)] = t
            return out

        def ln_block(src_ap, rows, pool_x, pool_h):
            """Load [128, D] f32 rows from DRAM, layernorm -> bf16 h."""
            xt = pool_x.tile([128, d.D], F32, tag="xt")
            nc.sync.dma_start(xt[:], src_ap[rows * 128:(rows + 1) * 128, :])
            rstd, nmr = _ln_stats(nc, small, xt, d.D, eps_t)
            h = pool_h.tile([128, d.D], BF16, tag="h")
            nc.scalar.activation(h[:], xt[:], AF.Identity, bias=nmr[:], scale=rstd[:])
            return xt, h

        def transpose_into(h_ap, dst_tile, dst_chunk0, tcol):
            """Transpose h_ap [128, D or chunk...]: for each 128-col chunk c,
            write h^T chunk into dst_tile[:, dst_chunk0+c, tcol*128:+128]."""
            nch = h_ap.shape[1] // 128
            for c0 in range(0, nch, 4):
                cn = min(4, nch - c0)
                pt = ptr.tile([128, 512], BF16, tag="pt")
                for c in range(cn):
                    nc.tensor.transpose(
                        pt[:, c * 128:(c + 1) * 128],
                        h_ap[:, (c0 + c) * 128:(c0 + c + 1) * 128], idt[:])
                nc.vector.tensor_copy(
                    dst_tile[:, dst_chunk0 + c0:dst_chunk0 + c0 + cn,
                             tcol * 128:(tcol + 1) * 128],
                    pt[:, :cn * 128].rearrange("p (a f) -> p a f", a=cn))

        for l in range(d.L):
            src = x0 if l == 0 else Xr
            stp = ctx.enter_context(tc.tile_pool(name=f"st{l}", bufs=1))
            s2 = stp.tile([128, d.R // 128, 2], F32, name=f"s2_{l}")

            # ---------------- attention: pass A + pass B1, per sequence ----
            with tc.tile_pool(name=f"wat{l}", bufs=1) as wat, \
                 tc.tile_pool(name=f"pha{l}", bufs=2) as htp, \
                 tc.tile_pool(name=f"wka{l}", bufs=2) as work, \
                 tc.tile_pool(name=f"xa{l}", bufs=3) as xin, \
                 tc.tile_pool(name=f"ha{l}", bufs=2) as hbuf, \
                 tc.tile_pool(name=f"oa{l}", bufs=3) as outp:
                wqS = wat.tile([128, nD, d.D], BF16, tag="wq")
                nc.sync.dma_start(wqS[:], wq_d[l].ap().rearrange("(a p) n -> p a n", p=128))
                wkS = wat.tile([128, nD, d.DH], BF16, tag="wk")
                nc.sync.dma_start(wkS[:], wk_d[l].ap().rearrange("(a p) n -> p a n", p=128))
                pkS = wat.tile([128, d.n_blk, d.K], BF16, tag="pk")
                nc.sync.dma_start(pkS[:], pk_d[l].ap().rearrange("(a p) k -> p a k", p=128))
                woS = wat.tile([128, nD, d.D], BF16, tag="wo")
                nc.sync.dma_start(woS[:], wo_d[l].ap().rearrange("(a p) n -> p a n", p=128))
                bias_sb = load_bias_rows(wat, l, ("cq", "bo"))

                hTs = {}

                def pass_a_block(b, t):
                    r = b * d.n_blk + t
                    if t == 0:
                        hTs[b] = htp.tile([128, nD, d.N], BF16, tag="hT",
                                          name=f"hT{l}_{b}")
                        cur_kv[b] = (pkv.tile([64, 64], F32, tag="kvT", name=f"kvTp{l}_{b}"),
                                     pkv.tile([64, 64], F32, tag="kv", name=f"kvp{l}_{b}"))
                    hT = hTs[b]
                    kvT_ps, kv_ps = cur_kv[b]
                    xt, h = ln_block(src.ap(), r, xin, hbuf)
                    transpose_into(h[:], hT, 0, t)
                    hk_ps = pmm.tile([128, d.DH], F32, tag="mm", name=f"hkps{l}_{r}")
                    for dc in range(nD):
                        nc.tensor.matmul(
                            hk_ps[:], hT[:, dc, t * 128:(t + 1) * 128],
                            wkS[:, dc, :], start=(dc == 0), stop=(dc == nD - 1))
                    hk = work.tile([128, d.DH], BF16, tag="hk")
                    nc.vector.tensor_copy(hk[:], hk_ps[:])
                    nc.tensor.matmul(kvT_ps[:], hk[:], pkS[:, t, :],
                                     start=(t == 0), stop=(t == d.n_blk - 1))
                    nc.tensor.matmul(kv_ps[:], pkS[:, t, :], hk[:],
                                     start=(t == 0), stop=(t == d.n_blk - 1))

                def bd_build(b):
                    kvT_ps, kv_ps = cur_kv.pop(b)
                    kvT = work.tile([64, 64], BF16, tag="kvT")
                    kv = work.tile([64, 64], BF16, tag="kv")
                    nc.vector.tensor_copy(kvT[:], kvT_ps[:])
                    nc.vector.tensor_copy(kv[:], kv_ps[:])
                    bdT_ps = pmm.tile([128, 128], F32, tag="mm", name="bdTps")
                    nc.tensor.matmul(bdT_ps[:, 0:64], ilo[:], kvT[:])
                    nc.tensor.matmul(bdT_ps[:, 64:128], ihi[:], kvT[:])
                    bdv_ps = pmm.tile([128, 128], F32, tag="mm", name="bdvps")
                    nc.tensor.matmul(bdv_ps[:, 0:64], ilo[:], kv[:])
                    nc.tensor.matmul(bdv_ps[:, 64:128], ihi[:], kv[:])
                    bdT = work.tile([128, 128], BF16, tag="bdT")
                    nc.vector.tensor_copy(bdT[:], bdT_ps[:])
                    bdv = work.tile([128, 128], BF16, tag="bdv")
                    nc.vector.tensor_copy(bdv[:], bdv_ps[:])
                    cur_bd[b] = (bdT, bdv)

                cur_kv = {}
                cur_bd = {}
                for t in range(d.n_blk):
                    pass_a_block(0, t)
                bd_build(0)
                for b in range(d.B_loc):
                    hT = hTs.pop(b)
                    bdT, bdv = cur_bd.pop(b)

                    # ---- pass B1: q, dots, softmax, o, Wo, residual -------
                    for c4 in range(d.n_chunk):
                        tok0 = c4 * 512
                        qT = work.tile([128, nD, 512], BF16, tag="qT")
                        for ncol in range(nD):
                            q_ps = pmm.tile([128, 512], F32, tag="mm")
                            for dc in range(nD):
                                nc.tensor.matmul(
                                    q_ps[:], wqS[:, dc, ncol * 128:(ncol + 1) * 128],
                                    hT[:, dc, tok0:tok0 + 512],
                                    start=(dc == 0),
                                    stop=(dc == nD - 1 and ("cq", l) not in bias_sb))
                            if ("cq", l) in bias_sb:
                                nc.tensor.matmul(
                                    q_ps[:], bias_sb[("cq", l)][:, ncol * 128:(ncol + 1) * 128],
                                    ones_bf[:], start=False, stop=True)
                            nc.vector.tensor_copy(qT[:, ncol, :], q_ps[:])

                        for tb in range(4):
                            t = c4 * 4 + tb
                            r = b * d.n_blk + t
                            # dots: two psum tiles cover 16 heads
                            dots_ps = [pmm.tile([128, 512], F32, tag="mm", name=f"dots{j}") for j in range(2)]
                            for c in range(nD):
                                nc.tensor.matmul(
                                    dots_ps[c // 4][:, (c % 4) * 128:(c % 4 + 1) * 128],
                                    qT[:, c, tb * 128:(tb + 1) * 128],
                                    bdT[:])
                            expt = work.tile([128, d.H * d.K], F32, tag="expt")
                            for j in range(2):
                                nc.scalar.activation(expt[:, j * 512:(j + 1) * 512],
                                                     dots_ps[j][:], AF.Exp,
                                                     scale=float(d.DH) ** -0.5)
                            se = small.tile([128, d.H], F32, tag="se")
                            nc.vector.reduce_sum(
                                se[:], expt[:].rearrange("p (h k) -> p h k", h=d.H),
                                axis=AX.X)
                            rse = small.tile([128, d.H], F32, tag="rse")
                            nc.vector.reciprocal(rse[:], se[:])
                            attn = work.tile([128, d.H * d.K], BF16, tag="attn")
                            for h_i in range(d.H):
                                nc.vector.tensor_scalar_mul(
                                    attn[:, h_i * 64:(h_i + 1) * 64],
                                    expt[:, h_i * 64:(h_i + 1) * 64],
                                    rse[:, h_i:h_i + 1])
                            attnT = work.tile([128, nD, 128], BF16, tag="attnT")
                            transpose_into(attn[:], attnT, 0, 0)
                            oT_ps = [pmm.tile([128, 512], F32, tag="mm", name=f"oTps{j}") for j in range(2)]
                            for c in range(nD):
                                nc.tensor.matmul(
                                    oT_ps[c // 4][:, (c % 4) * 128:(c % 4 + 1) * 128],
                                    bdv[:],
                                    attnT[:, c, :])
                            oT = work.tile([128, nD, 128], BF16, tag="oT")
                            for j in range(2):
                                nc.vector.tensor_copy(
                                    oT[:, j * 4:(j + 1) * 4, :],
                                    oT_ps[j][:].rearrange("p (a f) -> p a f", a=4))
                            # y = oT^T @ Wo (+bo) ; X' = X + y
                            xb = xin.tile([128, d.D], F32, tag="xres")
                            nc.sync.dma_start(xb[:], src.ap()[r * 128:(r + 1) * 128, :])
                            xp = outp.tile([128, d.D], F32, tag="xp")
                            for ncol in range(2):
                                y_ps = pmm.tile([128, 512], F32, tag="mm")
                                for dc in range(nD):
                                    nc.tensor.matmul(
                                        y_ps[:], oT[:, dc, :],
                                        woS[:, dc, ncol * 512:(ncol + 1) * 512],
                                        start=(dc == 0),
                                        stop=(dc == nD - 1 and ("bo", l) not in bias_sb))
                                if ("bo", l) in bias_sb:
                                    nc.tensor.matmul(
                                        y_ps[:], ones_bf[:, 0:128],
                                        bias_sb[("bo", l)][:, ncol * 512:(ncol + 1) * 512],
                                        start=False, stop=True)
                                nc.vector.scalar_tensor_tensor(
                                    xp[:, ncol * 512:(ncol + 1) * 512], y_ps[:], 1.0,
                                    xb[:, ncol * 512:(ncol + 1) * 512], ALU.mult, ALU.add)
                            _ln_stats(nc, small, xp, d.D, eps_t,
                                      out_rstd=s2[:, r, 0:1], out_nmr=s2[:, r, 1:2])
                            nc.sync.dma_start(Xp.ap()[r * 128:(r + 1) * 128, :], xp[:])
                        if b + 1 < d.B_loc:
                            for tb2 in range(4):
                                pass_a_block(b + 1, c4 * 4 + tb2)
                    if b + 1 < d.B_loc:
                        bd_build(b + 1)

            # ---------------- FF: pass B2, per 512-token chunk -------------
            with tc.tile_pool(name=f"wff{l}", bufs=1) as wff, \
                 tc.tile_pool(name=f"phf{l}", bufs=1) as htp, \
                 tc.tile_pool(name=f"xf{l}", bufs=2) as xin, \
                 tc.tile_pool(name=f"hf{l}", bufs=1) as hbuf, \
                 tc.tile_pool(name=f"of{l}", bufs=1) as outp:
                w1src = w1_d[l].ap().rearrange("(a p) n -> p a n", p=128)
                w1gs = []
                for g in range(4):
                    w1t = wff.tile([128, nD, d.FF // 4], BF16, tag=f"w1g{g}",
                                   name=f"w1_{l}_{g}")
                    nc.sync.dma_start(w1t[:], w1src[:, :, g * 1024:(g + 1) * 1024])
                    w1gs.append(w1t)
                w2src = w2_d[l].ap().rearrange("(a p) n -> p a n", p=128)
                w2gs = []
                for g in range(4):
                    w2t = wff.tile([128, nF // 4, d.D], BF16, tag=f"w2g{g}",
                                   name=f"w2_{l}_{g}")
                    nc.sync.dma_start(w2t[:], w2src[:, g * 8:(g + 1) * 8, :])
                    w2gs.append(w2t)
                bias_sb = load_bias_rows(wff, l, ("bu", "bz"))
                if l == d.L - 1:
                    for b in range(d.B_loc):
                        nc.vector.memset(mus[b][:], 0.0)

                for cg in range(d.R // 512):
                    h2T = htp.tile([128, nD, 512], BF16, tag="h2T")
                    for tb in range(4):
                        r = cg * 4 + tb
                        xt = xin.tile([128, d.D], F32, tag="xt")
                        nc.sync.dma_start(xt[:], Xp.ap()[r * 128:(r + 1) * 128, :])
                        h2 = hbuf.tile([128, d.D], BF16, tag="h")
                        nc.scalar.activation(h2[:], xt[:], AF.Identity,
                                             bias=s2[:, r, 1:2], scale=s2[:, r, 0:1])
                        transpose_into(h2[:], h2T, 0, tb)
                    uT = htp.tile([128, nF, 512], BF16, tag="uT")
                    for fc in range(nF):
                        u_ps = pmm.tile([128, 512], F32, tag="mm")
                        for dc in range(nD):
                            nc.tensor.matmul(
                                u_ps[:], w1gs[fc // 8][:, dc, (fc % 8) * 128:(fc % 8 + 1) * 128],
                                h2T[:, dc, :], start=(dc == 0),
                                stop=(dc == nD - 1 and ("bu", l) not in bias_sb))
                        if ("bu", l) in bias_sb:
                            nc.tensor.matmul(
                                u_ps[:], bias_sb[("bu", l)][:, fc * 128:(fc + 1) * 128],
                                ones_bf[:], start=False, stop=True)
                        nc.scalar.activation(uT[:, fc, :], u_ps[:], AF.Gelu)
                    if l == d.L - 1:
                        # pool trick: only token-sums of gelu(u) are needed
                        red = small.tile([128, nF], F32, tag="red")
                        nc.vector.reduce_sum(red[:], uT[:], axis=AX.X)
                        nc.vector.tensor_add(mus[cg // 4][:], mus[cg // 4][:], red[:])
                        continue
                    for tb in range(4):
                        r = cg * 4 + tb
                        xres = xin.tile([128, d.D], F32, tag="xres", bufs=1)
                        nc.sync.dma_start(xres[:], Xp.ap()[r * 128:(r + 1) * 128, :])
                        xo = outp.tile([128, d.D], F32, tag="xo")
                        for ncol in range(2):
                            z_ps = pmm.tile([128, 512], F32, tag="mm")
                            for fc in range(nF):
                                nc.tensor.matmul(
                                    z_ps[:], uT[:, fc, tb * 128:(tb + 1) * 128],
                                    w2gs[fc // 8][:, fc % 8, ncol * 512:(ncol + 1) * 512],
                                    start=(fc == 0),
                                    stop=(fc == nF - 1 and ("bz", l) not in bias_sb))
                            if ("bz", l) in bias_sb:
                                nc.tensor.matmul(
                                    z_ps[:], ones_bf[:, 0:128],
                                    bias_sb[("bz", l)][:, ncol * 512:(ncol + 1) * 512],
                                    start=False, stop=True)
                            nc.vector.scalar_tensor_tensor(
                                xo[:, ncol * 512:(ncol + 1) * 512], z_ps[:], 1.0,
                                xres[:, ncol * 512:(ncol + 1) * 512],
                                ALU.mult, ALU.add)
                        nc.sync.dma_start(Xr.ap()[r * 128:(r + 1) * 128, :], xo[:])

        # ---------------- final: mean over tokens (+ pooled W2), layernorm -
        fin = ctx.enter_context(tc.tile_pool(name="fin", bufs=2))
        gt = fin.tile([1, d.D], F32, tag="lnfg", bufs=1)
        nc.sync.dma_start(gt[:], lnfg.ap())
        bt = fin.tile([1, d.D], F32, tag="lnfb", bufs=1)
        nc.sync.dma_start(bt[:], lnfb.ap())
        wfp = ctx.enter_context(tc.tile_pool(name="w2fin", bufs=1))
        w2f = wfp.tile([128, nF, d.D], BF16, tag="w2f", name="w2fin")
        nc.sync.dma_start(w2f[:], w2_d[d.L - 1].ap().rearrange("(a p) n -> p a n", p=128))
        for b in range(d.B_loc):
            e_ps = [pmm.tile([1, 512], F32, tag="mm", name=f"eps{j}") for j in range(2)]
            for t in range(d.n_blk):
                r = b * d.n_blk + t
                xb = fin.tile([128, d.D], F32, tag="xt")
                nc.sync.dma_start(xb[:], Xp.ap()[r * 128:(r + 1) * 128, :])
                for j in range(2):
                    nc.tensor.matmul(e_ps[j][:], ones_f32[:], xb[:, j * 512:(j + 1) * 512],
                                     start=(t == 0), stop=(t == d.n_blk - 1))
            mu_bf = fin.tile([128, 32], BF16, tag="mubf")
            nc.scalar.mul(mu_bf[:], mus[b][:], 1.0 / d.N)
            z_ps = [pmm.tile([1, 512], F32, tag="mm", name=f"zps{j}") for j in range(2)]
            for j in range(2):
                for fc in range(nF):
                    nc.tensor.matmul(z_ps[j][:], mu_bf[:, fc:fc + 1],
                                     w2f[:, fc, j * 512:(j + 1) * 512],
                                     start=(fc == 0), stop=(fc == nF - 1))
            emb = fin.tile([1, d.D], F32, tag="emb")
            for j in range(2):
                nc.scalar.mul(emb[:, j * 512:(j + 1) * 512], e_ps[j][:], 1.0 / d.N)
                nc.vector.tensor_add(emb[:, j * 512:(j + 1) * 512],
                                     emb[:, j * 512:(j + 1) * 512], z_ps[j][:])
            rstd, nmr = _ln_stats(nc, small, emb, d.D, eps_t)
            nrm = fin.tile([1, d.D], F32, tag="nrm")
            nc.scalar.activation(nrm[:], emb[:], AF.Identity, bias=nmr[:], scale=rstd[:])
            ot = fin.tile([1, d.D], F32, tag="ot")
            nc.vector.tensor_mul(ot[:], nrm[:], gt[:])
            nc.vector.tensor_add(ot[:], ot[:], bt[:])
            nc.sync.dma_start(out.ap()[b:b + 1, :], ot[:])

    nc.compile()
    return nc


_CACHE = {}


def _to_bf16(a):
    return np.asarray(a, dtype=np.float32).astype(ml_dtypes.bfloat16)


def prepare_inputs(dims: Dims, n_cores, token_ids, token_emb, pos_emb, ln1_g, ln1_b,
                   Wq, Wk, Pk, Wo, bo, ln2_g, ln2_b, W1, b1, W2, b2, lnf_g, lnf_b):
    d = dims
    token_ids = np.asarray(token_ids)
    token_emb = np.asarray(token_emb, dtype=np.float32)
    pos_emb = np.asarray(pos_emb, dtype=np.float32)

    x_all = token_emb[token_ids[:, :d.N]] + pos_emb[None, :d.N, :]  # [B, N, D]
    B = token_ids.shape[0]
    assert B == n_cores * d.B_loc

    biases = {}
    shared = {}
    for l in range(d.L):
        g1 = np.asarray(ln1_g[l], np.float32)
        b1l = np.asarray(ln1_b[l], np.float32)
        g2 = np.asarray(ln2_g[l], np.float32)
        b2l = np.asarray(ln2_b[l], np.float32)
        Wql = np.asarray(Wq[l], np.float32)
        Wkl = np.asarray(Wk[l], np.float32)
        W1l = np.asarray(W1[l], np.float32)
        shared[f"wq{l}"] = _to_bf16(g1[:, None] * Wql)
        shared[f"wk{l}"] = _to_bf16(g1[:, None] * Wkl)
        shared[f"pk{l}"] = _to_bf16(np.asarray(Pk[l])[:d.N])
        shared[f"wo{l}"] = _to_bf16(Wo[l])
        shared[f"w1{l}"] = _to_bf16(g2[:, None] * W1l)
        shared[f"w2{l}"] = _to_bf16(W2[l])

        def nz(v):
            v = np.asarray(v, np.float32)
            return v if np.any(v != 0) else None

        cq = nz(b1l @ Wql)
        ck = b1l @ Wkl                                   # [DH]
        colsum = np.asarray(Pk[l], np.float32)[:d.N].sum(axis=0)   # [K]
        kvc = nz(np.outer(colsum, ck))                   # [K, DH]
        bul = nz(b2l @ W1l + np.asarray(b1[l], np.float32))
        bol = nz(bo[l])
        bzl = nz(b2[l])
        biases[("cq", l)] = _to_bf16(cq) if cq is not None else None
        biases[("kvc", l)] = kvc.astype(np.float32) if kvc is not None else None
        biases[("kvtc", l)] = kvc.T.astype(np.float32).copy() if kvc is not None else None
        biases[("bu", l)] = _to_bf16(bul) if bul is not None else None
        biases[("bo", l)] = _to_bf16(bol) if bol is not None else None
        biases[("bz", l)] = _to_bf16(bzl) if bzl is not None else None

    lnf_g_rep = np.asarray(lnf_g, np.float32).reshape(1, d.D).copy()
    lnf_b_rep = np.asarray(lnf_b, np.float32).reshape(1, d.D).copy()

    in_maps = []
    for c in range(n_cores):
        m = dict(shared)
        m["x0"] = np.ascontiguousarray(
            x_all[c * d.B_loc:(c + 1) * d.B_loc].reshape(d.R, d.D), dtype=np.float32)
        m["lnfg"] = lnf_g_rep
        m["lnfb"] = lnf_b_rep
        for key, v in biases.items():
            if v is not None:
                m[f"{key[0]}{key[1]}"] = v
        in_maps.append(m)
    return in_maps, biases


def run(dims: Dims, n_cores, inputs, trace=False, tmpdir=None):
    in_maps, biases = prepare_inputs(dims, n_cores, **inputs)
    ck = (dims, n_cores, tuple(sorted(k for k, v in biases.items() if v is not None)))
    if ck not in _CACHE:
        _CACHE[ck] = build(dims, n_cores, biases)
    nc = _CACHE[ck]
    res = run_bass_kernel_spmd(nc, in_maps, list(range(n_cores)), trace=trace,
                               tmpdir=tmpdir)
    outs = np.concatenate([res.results[i]["out"] for i in range(n_cores)], axis=0)
    return outs, res


def kernel(**inputs) -> np.ndarray:
    out, _ = run(Dims(), 8, inputs)
    return out.astype(np.float32)



# revision 12
# speedup vs baseline: 1.1070x; 1.0213x over previous
"""Linformer text encoder on 8 TRN2 NeuronCores.

Sharding: pure data-parallel over batch (32 seqs -> 4 per core), weights
replicated, no collectives. Host does the embedding gather (cheaper to stage
32MB/core of gathered rows than 205MB/core of table) and folds LN gamma/beta
into the following projection weights (exact math). Device does everything
else in bf16 matmuls with f32 accumulation.

Self-contained: hardcodes all shapes from the problem spec.
"""

import sys

sys.path.insert(0, "/opt/trn_rl_repo")

from contextlib import ExitStack
from dataclasses import dataclass

import ml_dtypes
import numpy as np

import concourse.bass as bass
import concourse.tile as tile
from concourse import bacc, mybir
from concourse.bass_utils import run_bass_kernel_spmd
from concourse.masks import make_identity

F32 = mybir.dt.float32
BF16 = mybir.dt.bfloat16
AF = mybir.ActivationFunctionType
ALU = mybir.AluOpType
AX = mybir.AxisListType

EPS = 1e-5


@dataclass(frozen=True)
class Dims:
    B_loc: int = 4      # sequences per core
    N: int = 2048       # tokens per sequence
    D: int = 1024
    H: int = 16
    DH: int = 64
    K: int = 64
    FF: int = 4096
    L: int = 4

    @property
    def R(self):
        return self.B_loc * self.N

    @property
    def n_blk(self):        # 128-token blocks per sequence
        return self.N // 128

    @property
    def n_chunk(self):      # 512-token chunks per sequence
        return self.N // 512


def _ln_stats(nc, pool, xt, d, eps_t, out_rstd=None, out_nmr=None):
    """Token-major LN stats for xt [128, d] f32.
    Returns (rstd [P,1] f32, neg_mean_rstd [P,1] f32).
    Uses exp(-0.5*ln(var+eps)) so the whole kernel stays inside the
    natural_log_exp activation-table set (no LUT reloads)."""
    P = xt.shape[0]
    ngrp = d // 512
    bns = pool.tile([P, ngrp, 6], F32, tag="bns")
    for g in range(ngrp):
        nc.vector.bn_stats(bns[:, g, :], xt[:, g * 512:(g + 1) * 512])
    mv = pool.tile([P, 2], F32, tag="mv")
    nc.vector.bn_aggr(mv[:], bns[:])
    lnv = pool.tile([P, 1], F32, tag="lnv")
    nc.scalar.activation(lnv[:], mv[:, 1:2], AF.Ln, bias=eps_t[:P, :])
    rstd = out_rstd if out_rstd is not None else pool.tile([P, 1], F32, tag="rstd")
    nc.scalar.activation(rstd[:], lnv[:], AF.Exp, scale=-0.5)
    nmr = out_nmr if out_nmr is not None else pool.tile([P, 1], F32, tag="nmr")
    # nmr = (mean * -1) * rstd
    nc.vector.scalar_tensor_tensor(nmr[:], mv[:, 0:1], -1.0, rstd[:], ALU.mult, ALU.mult)
    return rstd, nmr


def build(dims: Dims, n_cores: int, biases):
    """Emit the full per-core program. `biases` is a dict of host numpy
    vectors (cq, kvc, kvtc, bu, bo, bz per layer) or None entries when zero."""
    d = dims
    nc = bacc.Bacc("TRN2", target_bir_lowering=False, debug=False,
                   num_devices=n_cores, enable_asserts=False)

    x0 = nc.dram_tensor("x0", [d.R, d.D], F32, kind="ExternalInput")
    wq_d = [nc.dram_tensor(f"wq{l}", [d.D, d.D], BF16, kind="ExternalInput") for l in range(d.L)]
    wk_d = [nc.dram_tensor(f"wk{l}", [d.D, d.DH], BF16, kind="ExternalInput") for l in range(d.L)]
    pk_d = [nc.dram_tensor(f"pk{l}", [d.N, d.K], BF16, kind="ExternalInput") for l in range(d.L)]
    wo_d = [nc.dram_tensor(f"wo{l}", [d.D, d.D], BF16, kind="ExternalInput") for l in range(d.L)]
    w1_d = [nc.dram_tensor(f"w1{l}", [d.D, d.FF], BF16, kind="ExternalInput") for l in range(d.L)]
    w2_d = [nc.dram_tensor(f"w2{l}", [d.FF, d.D], BF16, kind="ExternalInput") for l in range(d.L)]
    lnfg = nc.dram_tensor("lnfg", [1, d.D], F32, kind="ExternalInput")
    lnfb = nc.dram_tensor("lnfb", [1, d.D], F32, kind="ExternalInput")
    out = nc.dram_tensor("out", [d.B_loc, d.D], F32, kind="ExternalOutput")

    bias_d = {}
    for l in range(d.L):
        for nm in ("cq", "kvc", "kvtc", "bu", "bo", "bz"):
            if biases and biases.get((nm, l)) is not None:
                arr = biases[(nm, l)]
                dt = F32 if nm in ("kvc", "kvtc") else BF16
                bias_d[(nm, l)] = nc.dram_tensor(
                    f"{nm}{l}", list(arr.shape), dt, kind="ExternalInput")

    Xp = nc.dram_tensor("Xp", [d.R, d.D], F32)   # post-attention residual
    Xr = nc.dram_tensor("Xr", [d.R, d.D], F32)   # post-FF residual

    nD = d.D // 128       # 8 feature chunks
    nF = d.FF // 128      # 32 ff chunks

    with ExitStack() as ctx:
        tc = ctx.enter_context(tile.TileContext(nc))
        const = ctx.enter_context(tc.tile_pool(name="const", bufs=1))
        small = ctx.enter_context(tc.tile_pool(name="small", bufs=4))
        mup = ctx.enter_context(tc.tile_pool(name="mup", bufs=1))
        mup = ctx.enter_context(tc.tile_pool(name="mup", bufs=1))
        pmm = ctx.enter_context(tc.tile_pool(name="pmm", bufs=4, space="PSUM"))
        ptr = ctx.enter_context(tc.tile_pool(name="ptr", bufs=2, space="PSUM"))
        pkv = ctx.enter_context(tc.tile_pool(name="pkv", bufs=1, space="PSUM"))

        mus = [mup.tile([128, 32], F32, tag=f"mu{b}", name=f"mu{b}")
               for b in range(4)]
        idt = const.tile([128, 128], BF16)
        make_identity(nc, idt[:])
        ones_bf = const.tile([1, 512], BF16)
        nc.vector.memset(ones_bf[:], 1.0)
        ones_f32 = const.tile([128, 1], F32)
        nc.vector.memset(ones_f32[:], 1.0)
        eps_t = const.tile([128, 1], F32)
        nc.vector.memset(eps_t[:], EPS)
        ilo = const.tile([64, 128], BF16)
        nc.vector.memset(ilo[:], 0.0)
        make_identity(nc, ilo[:, 0:64])
        ihi = const.tile([64, 128], BF16)
        nc.vector.memset(ihi[:], 0.0)
        make_identity(nc, ihi[:, 64:128])

        def load_bias_rows(pool, l, names):
            out = {}
            for nm in names:
                if (nm, l) in bias_d:
                    dram = bias_d[(nm, l)]
                    t = pool.tile([1, dram.shape[0]], BF16, tag=f"b{nm}",
                                  name=f"b{nm}{l}")
                    nc.sync.dma_start(t[:], dram.ap()[None, :])
                    out[(nm, l)] = t
            return out

        def ln_block(src_ap, rows, pool_x, pool_h):
            """Load [128, D] f32 rows from DRAM, layernorm -> bf16 h."""
            xt = pool_x.tile([128, d.D], F32, tag="xt")
            nc.sync.dma_start(xt[:], src_ap[rows * 128:(rows + 1) * 128, :])
            rstd, nmr = _ln_stats(nc, small, xt, d.D, eps_t)
            h = pool_h.tile([128, d.D], BF16, tag="h")
            nc.scalar.activation(h[:], xt[:], AF.Identity, bias=nmr[:], scale=rstd[:])
            return xt, h

        def transpose_into(h_ap, dst_tile, dst_chunk0, tcol):
            """Transpose h_ap [128, D or chunk...]: for each 128-col chunk c,
            write h^T chunk into dst_tile[:, dst_chunk0+c, tcol*128:+128]."""
            nch = h_ap.shape[1] // 128
            for c0 in range(0, nch, 4):
                cn = min(4, nch - c0)
                pt = ptr.tile([128, 512], BF16, tag="pt")
                for c in range(cn):
                    nc.tensor.transpose(
                        pt[:, c * 128:(c + 1) * 128],
                        h_ap[:, (c0 + c) * 128:(c0 + c + 1) * 128], idt[:])
                nc.vector.tensor_copy(
                    dst_tile[:, dst_chunk0 + c0:dst_chunk0 + c0 + cn,
                             tcol * 128:(tcol + 1) * 128],
                    pt[:, :cn * 128].rearrange("p (a f) -> p a f", a=cn))

        for l in range(d.L):
            src = x0 if l == 0 else Xr
            stp = ctx.enter_context(tc.tile_pool(name=f"st{l}", bufs=1))
            s2 = stp.tile([128, d.R // 128, 2], F32, name=f"s2_{l}")

            # ---------------- attention: pass A + pass B1, per sequence ----
            with tc.tile_pool(name=f"wat{l}", bufs=1) as wat, \
                 tc.tile_pool(name=f"pha{l}", bufs=2) as htp, \
                 tc.tile_pool(name=f"wka{l}", bufs=2) as work, \
                 tc.tile_pool(name=f"xa{l}", bufs=3) as xin, \
                 tc.tile_pool(name=f"ha{l}", bufs=2) as hbuf, \
                 tc.tile_pool(name=f"oa{l}", bufs=3) as outp:
                wqS = wat.tile([128, nD, d.D], BF16, tag="wq")
                nc.sync.dma_start(wqS[:], wq_d[l].ap().rearrange("(a p) n -> p a n", p=128))
                wkS = wat.tile([128, nD, d.DH], BF16, tag="wk")
                nc.sync.dma_start(wkS[:], wk_d[l].ap().rearrange("(a p) n -> p a n", p=128))
                pkS = wat.tile([128, d.n_blk, d.K], BF16, tag="pk")
                nc.sync.dma_start(pkS[:], pk_d[l].ap().rearrange("(a p) k -> p a k", p=128))
                woS = wat.tile([128, nD, d.D], BF16, tag="wo")
                nc.sync.dma_start(woS[:], wo_d[l].ap().rearrange("(a p) n -> p a n", p=128))
                bias_sb = load_bias_rows(wat, l, ("cq", "bo"))

                hTs = {}

                def pass_a_block(b, t):
                    r = b * d.n_blk + t
                    if t == 0:
                        hTs[b] = htp.tile([128, nD, d.N], BF16, tag="hT",
                                          name=f"hT{l}_{b}")
                        cur_kv[b] = (pkv.tile([64, 64], F32, tag="kvT", name=f"kvTp{l}_{b}"),
                                     pkv.tile([64, 64], F32, tag="kv", name=f"kvp{l}_{b}"))
                    hT = hTs[b]
                    kvT_ps, kv_ps = cur_kv[b]
                    xt, h = ln_block(src.ap(), r, xin, hbuf)
                    transpose_into(h[:], hT, 0, t)
                    hk_ps = pmm.tile([128, d.DH], F32, tag="mm", name=f"hkps{l}_{r}")
                    for dc in range(nD):
                        nc.tensor.matmul(
                            hk_ps[:], hT[:, dc, t * 128:(t + 1) * 128],
                            wkS[:, dc, :], start=(dc == 0), stop=(dc == nD - 1))
                    hk = work.tile([128, d.DH], BF16, tag="hk")
                    nc.vector.tensor_copy(hk[:], hk_ps[:])
                    nc.tensor.matmul(kvT_ps[:], hk[:], pkS[:, t, :],
                                     start=(t == 0), stop=(t == d.n_blk - 1))
                    nc.tensor.matmul(kv_ps[:], pkS[:, t, :], hk[:],
                                     start=(t == 0), stop=(t == d.n_blk - 1))

                def bd_build(b):
                    kvT_ps, kv_ps = cur_kv.pop(b)
                    kvT = work.tile([64, 64], BF16, tag="kvT")
                    kv = work.tile([64, 64], BF16, tag="kv")
                    nc.vector.tensor_copy(kvT[:], kvT_ps[:])
                    nc.vector.tensor_copy(kv[:], kv_ps[:])
                    bdT_ps = pmm.tile([128, 128], F32, tag="mm", name="bdTps")
                    nc.tensor.matmul(bdT_ps[:, 0:64], ilo[:], kvT[:])
                    nc.tensor.matmul(bdT_ps[:, 64:128], ihi[:], kvT[:])
                    bdv_ps = pmm.tile([128, 128], F32, tag="mm", name="bdvps")
                    nc.tensor.matmul(bdv_ps[:, 0:64], ilo[:], kv[:])
                    nc.tensor.matmul(bdv_ps[:, 64:128], ihi[:], kv[:])
                    bdT = work.tile([128, 128], BF16, tag="bdT")
                    nc.vector.tensor_copy(bdT[:], bdT_ps[:])
                    bdv = work.tile([128, 128], BF16, tag="bdv")
                    nc.vector.tensor_copy(bdv[:], bdv_ps[:])
                    cur_bd[b] = (bdT, bdv)

                cur_kv = {}
                cur_bd = {}
                for t in range(d.n_blk):
                    pass_a_block(0, t)
                bd_build(0)
                for b in range(d.B_loc):
                    hT = hTs.pop(b)
                    bdT, bdv = cur_bd.pop(b)

                    # ---- pass B1: q, dots, softmax, o, Wo, residual -------
                    for c4 in range(d.n_chunk):
                        tok0 = c4 * 512
                        qT = work.tile([128, nD, 512], BF16, tag="qT")
                        for ncol in range(nD):
                            q_ps = pmm.tile([128, 512], F32, tag="mm")
                            for dc in range(nD):
                                nc.tensor.matmul(
                                    q_ps[:], wqS[:, dc, ncol * 128:(ncol + 1) * 128],
                                    hT[:, dc, tok0:tok0 + 512],
                                    start=(dc == 0),
                                    stop=(dc == nD - 1 and ("cq", l) not in bias_sb))
                            if ("cq", l) in bias_sb:
                                nc.tensor.matmul(
                                    q_ps[:], bias_sb[("cq", l)][:, ncol * 128:(ncol + 1) * 128],
                                    ones_bf[:], start=False, stop=True)
                            nc.vector.tensor_copy(qT[:, ncol, :], q_ps[:])

                        for tb in range(4):
                            t = c4 * 4 + tb
                            r = b * d.n_blk + t
                            # dots: two psum tiles cover 16 heads
                            dots_ps = [pmm.tile([128, 512], F32, tag="mm", name=f"dots{j}") for j in range(2)]
                            for c in range(nD):
                                nc.tensor.matmul(
                                    dots_ps[c // 4][:, (c % 4) * 128:(c % 4 + 1) * 128],
                                    qT[:, c, tb * 128:(tb + 1) * 128],
                                    bdT[:])
                            expt = work.tile([128, d.H * d.K], F32, tag="expt")
                            for j in range(2):
                                nc.scalar.activation(expt[:, j * 512:(j + 1) * 512],
                                                     dots_ps[j][:], AF.Exp,
                                                     scale=float(d.DH) ** -0.5)
                            se = small.tile([128, d.H], F32, tag="se")
                            nc.vector.reduce_sum(
                                se[:], expt[:].rearrange("p (h k) -> p h k", h=d.H),
                                axis=AX.X)
                            rse = small.tile([128, d.H], F32, tag="rse")
                            nc.vector.reciprocal(rse[:], se[:])
                            attn = work.tile([128, d.H * d.K], BF16, tag="attn")
                            for h_i in range(d.H):
                                nc.vector.tensor_scalar_mul(
                                    attn[:, h_i * 64:(h_i + 1) * 64],
                                    expt[:, h_i * 64:(h_i + 1) * 64],
                                    rse[:, h_i:h_i + 1])
                            attnT = work.tile([128, nD, 128], BF16, tag="attnT")
                            transpose_into(attn[:], attnT, 0, 0)
                            oT_ps = [pmm.tile([128, 512], F32, tag="mm", name=f"oTps{j}") for j in range(2)]
                            for c in range(nD):
                                nc.tensor.matmul(
                                    oT_ps[c // 4][:, (c % 4) * 128:(c % 4 + 1) * 128],
                                    bdv[:],
                                    attnT[:, c, :])
                            oT = work.tile([128, nD, 128], BF16, tag="oT")
                            for j in range(2):
                                nc.vector.tensor_copy(
                                    oT[:, j * 4:(j + 1) * 4, :],
                                    oT_ps[j][:].rearrange("p (a f) -> p a f", a=4))
                            # y = oT^T @ Wo (+bo) ; X' = X + y
                            xb = xin.tile([128, d.D], F32, tag="xres")
                            nc.sync.dma_start(xb[:], src.ap()[r * 128:(r + 1) * 128, :])
                            xp = outp.tile([128, d.D], F32, tag="xp")
                            for ncol in range(2):
                                y_ps = pmm.tile([128, 512], F32, tag="mm")
                                for dc in range(nD):
                                    nc.tensor.matmul(
                                        y_ps[:], oT[:, dc, :],
                                        woS[:, dc, ncol * 512:(ncol + 1) * 512],
                                        start=(dc == 0),
                                        stop=(dc == nD - 1 and ("bo", l) not in bias_sb))
                                if ("bo", l) in bias_sb:
                                    nc.tensor.matmul(
                                        y_ps[:], ones_bf[:, 0:128],
                                        bias_sb[("bo", l)][:, ncol * 512:(ncol + 1) * 512],
                                        start=False, stop=True)
                                nc.vector.scalar_tensor_tensor(
                                    xp[:, ncol * 512:(ncol + 1) * 512], y_ps[:], 1.0,
                                    xb[:, ncol * 512:(ncol + 1) * 512], ALU.mult, ALU.add)
                            _ln_stats(nc, small, xp, d.D, eps_t,
                                      out_rstd=s2[:, r, 0:1], out_nmr=s2[:, r, 1:2])
                            nc.sync.dma_start(Xp.ap()[r * 128:(r + 1) * 128, :], xp[:])
                        if b + 1 < d.B_loc:
                            for tb2 in range(4):
                                pass_a_block(b + 1, c4 * 4 + tb2)
                    if b + 1 < d.B_loc:
                        bd_build(b + 1)

            # ---------------- FF: pass B2, per 512-token chunk -------------
            with tc.tile_pool(name=f"wff{l}", bufs=1) as wff, \
                 tc.tile_pool(name=f"phf{l}", bufs=1) as htp, \
                 tc.tile_pool(name=f"h2p{l}", bufs=2) as h2p, \
                 tc.tile_pool(name=f"xf{l}", bufs=2) as xin, \
                 tc.tile_pool(name=f"hf{l}", bufs=1) as hbuf, \
                 tc.tile_pool(name=f"of{l}", bufs=1) as outp:
                def prep_chunk(cg):
                    h2T = h2p.tile([128, nD, 512], BF16, tag="h2T",
                                   name=f"h2T{l}_{cg}")
                    for tb in range(4):
                        r = cg * 4 + tb
                        xt = xin.tile([128, d.D], F32, tag="xt")
                        nc.sync.dma_start(xt[:], Xp.ap()[r * 128:(r + 1) * 128, :])
                        h2 = hbuf.tile([128, d.D], BF16, tag="h")
                        nc.scalar.activation(h2[:], xt[:], AF.Identity,
                                             bias=s2[:, r, 1:2], scale=s2[:, r, 0:1])
                        transpose_into(h2[:], h2T, 0, tb)
                    return h2T
                h2T_next = prep_chunk(0)
                w1src = w1_d[l].ap().rearrange("(a p) n -> p a n", p=128)
                w1gs = []
                for g in range(4):
                    w1t = wff.tile([128, nD, d.FF // 4], BF16, tag=f"w1g{g}",
                                   name=f"w1_{l}_{g}")
                    nc.sync.dma_start(w1t[:], w1src[:, :, g * 1024:(g + 1) * 1024])
                    w1gs.append(w1t)
                w2src = w2_d[l].ap().rearrange("(a p) n -> p a n", p=128)
                w2gs = []
                for g in range(4):
                    w2t = wff.tile([128, nF // 4, d.D], BF16, tag=f"w2g{g}",
                                   name=f"w2_{l}_{g}")
                    nc.sync.dma_start(w2t[:], w2src[:, g * 8:(g + 1) * 8, :])
                    w2gs.append(w2t)
                bias_sb = load_bias_rows(wff, l, ("bu", "bz"))
                if l == d.L - 1:
                    for b in range(d.B_loc):
                        nc.vector.memset(mus[b][:], 0.0)

                for cg in range(d.R // 512):
                    h2T = h2T_next
                    if cg + 1 < d.R // 512:
                        h2T_next = prep_chunk(cg + 1)
                    uT = htp.tile([128, nF, 512], BF16, tag="uT")
                    for fc in range(nF):
                        u_ps = pmm.tile([128, 512], F32, tag="mm")
                        for dc in range(nD):
                            nc.tensor.matmul(
                                u_ps[:], w1gs[fc // 8][:, dc, (fc % 8) * 128:(fc % 8 + 1) * 128],
                                h2T[:, dc, :], start=(dc == 0),
                                stop=(dc == nD - 1 and ("bu", l) not in bias_sb))
                        if ("bu", l) in bias_sb:
                            nc.tensor.matmul(
                                u_ps[:], bias_sb[("bu", l)][:, fc * 128:(fc + 1) * 128],
                                ones_bf[:], start=False, stop=True)
                        nc.scalar.activation(uT[:, fc, :], u_ps[:], AF.Gelu)
                    if l == d.L - 1:
                        # pool trick: only token-sums of gelu(u) are needed
                        red = small.tile([128, nF], F32, tag="red")
                        nc.vector.reduce_sum(red[:], uT[:], axis=AX.X)
                        nc.vector.tensor_add(mus[cg // 4][:], mus[cg // 4][:], red[:])
                        continue
                    for tb in range(4):
                        r = cg * 4 + tb
                        xres = xin.tile([128, d.D], F32, tag="xres", bufs=1)
                        nc.sync.dma_start(xres[:], Xp.ap()[r * 128:(r + 1) * 128, :])
                        xo = outp.tile([128, d.D], F32, tag="xo")
                        for ncol in range(2):
                            z_ps = pmm.tile([128, 512], F32, tag="mm")
                            for fc in range(nF):
                                nc.tensor.matmul(
                                    z_ps[:], uT[:, fc, tb * 128:(tb + 1) * 128],
                                    w2gs[fc // 8][:, fc % 8, ncol * 512:(ncol + 1) * 512],
                                    start=(fc == 0),
                                    stop=(fc == nF - 1 and ("bz", l) not in bias_sb))
                            if ("bz", l) in bias_sb:
                                nc.tensor.matmul(
                                    z_ps[:], ones_bf[:, 0:128],
                                    bias_sb[("bz", l)][:, ncol * 512:(ncol + 1) * 512],
                                    start=False, stop=True)
                            nc.vector.scalar_tensor_tensor(
                                xo[:, ncol * 512:(ncol + 1) * 512], z_ps[:], 1.0,
                                xres[:, ncol * 512:(ncol + 1) * 512],
                                ALU.mult, ALU.add)
                        nc.sync.dma_start(Xr.ap()[r * 128:(r + 1) * 128, :], xo[:])

        # ---------------- final: mean over tokens (+ pooled W2), layernorm -
        fin = ctx.enter_context(tc.tile_pool(name="fin", bufs=2))
        gt = fin.tile([1, d.D], F32, tag="lnfg", bufs=1)
        nc.sync.dma_start(gt[:], lnfg.ap())
        bt = fin.tile([1, d.D], F32, tag="lnfb", bufs=1)
        nc.sync.dma_start(bt[:], lnfb.ap())
        wfp = ctx.enter_context(tc.tile_pool(name="w2fin", bufs=1))
        w2f = wfp.tile([128, nF, d.D], BF16, tag="w2f", name="w2fin")
        nc.sync.dma_start(w2f[:], w2_d[d.L - 1].ap().rearrange("(a p) n -> p a n", p=128))
        for b in range(d.B_loc):
            e_ps = [pmm.tile([1, 512], F32, tag="mm", name=f"eps{j}") for j in range(2)]
            for t in range(d.n_blk):
                r = b * d.n_blk + t
                xb = fin.tile([128, d.D], F32, tag="xt")
                nc.sync.dma_start(xb[:], Xp.ap()[r * 128:(r + 1) * 128, :])
                for j in range(2):
                    nc.tensor.matmul(e_ps[j][:], ones_f32[:], xb[:, j * 512:(j + 1) * 512],
                                     start=(t == 0), stop=(t == d.n_blk - 1))
            mu_bf = fin.tile([128, 32], BF16, tag="mubf")
            nc.scalar.mul(mu_bf[:], mus[b][:], 1.0 / d.N)
            z_ps = [pmm.tile([1, 512], F32, tag="mm", name=f"zps{j}") for j in range(2)]
            for j in range(2):
                for fc in range(nF):
                    nc.tensor.matmul(z_ps[j][:], mu_bf[:, fc:fc + 1],
                                     w2f[:, fc, j * 512:(j + 1) * 512],
                                     start=(fc == 0), stop=(fc == nF - 1))
            emb = fin.tile([1, d.D], F32, tag="emb")
            for j in range(2):
                nc.scalar.mul(emb[:, j * 512:(j + 1) * 512], e_ps[j][:], 1.0 / d.N)
                nc.vector.tensor_add(emb[:, j * 512:(j + 1) * 512],
                                     emb[:, j * 512:(j + 1) * 512], z_ps[j][:])
            rstd, nmr = _ln_stats(nc, small, emb, d.D, eps_t)
            nrm = fin.tile([1, d.D], F32, tag="nrm")
            nc.scalar.activation(nrm[:], emb[:], AF.Identity, bias=nmr[:], scale=rstd[:])
            ot = fin.tile([1, d.D], F32, tag="ot")
            nc.vector.tensor_mul(ot[:], nrm[:], gt[:])
            nc.vector.tensor_add(ot[:], ot[:], bt[:])
            nc.sync.dma_start(out.ap()[b:b + 1, :], ot[:])

    nc.compile()
    return nc


_CACHE = {}


def _to_bf16(a):
    return np.asarray(a, dtype=np.float32).astype(ml_dtypes.bfloat16)


def prepare_inputs(dims: Dims, n_cores, token_ids, token_emb, pos_emb, ln1_g, ln1_b,
                   Wq, Wk, Pk, Wo, bo, ln2_g, ln2_b, W1, b1, W2, b2, lnf_g, lnf_b):
    d = dims
    token_ids = np.asarray(token_ids)
    token_emb = np.asarray(token_emb, dtype=np.float32)
    pos_emb = np.asarray(pos_emb, dtype=np.float32)

    x_all = token_emb[token_ids[:, :d.N]] + pos_emb[None, :d.N, :]  # [B, N, D]
    B = token_ids.shape[0]
    assert B == n_cores * d.B_loc

    biases = {}
    shared = {}
    for l in range(d.L):
        g1 = np.asarray(ln1_g[l], np.float32)
        b1l = np.asarray(ln1_b[l], np.float32)
        g2 = np.asarray(ln2_g[l], np.float32)
        b2l = np.asarray(ln2_b[l], np.float32)
        Wql = np.asarray(Wq[l], np.float32)
        Wkl = np.asarray(Wk[l], np.float32)
        W1l = np.asarray(W1[l], np.float32)
        shared[f"wq{l}"] = _to_bf16(g1[:, None] * Wql)
        shared[f"wk{l}"] = _to_bf16(g1[:, None] * Wkl)
        shared[f"pk{l}"] = _to_bf16(np.asarray(Pk[l])[:d.N])
        shared[f"wo{l}"] = _to_bf16(Wo[l])
        shared[f"w1{l}"] = _to_bf16(g2[:, None] * W1l)
        shared[f"w2{l}"] = _to_bf16(W2[l])

        def nz(v):
            v = np.asarray(v, np.float32)
            return v if np.any(v != 0) else None

        cq = nz(b1l @ Wql)
        ck = b1l @ Wkl                                   # [DH]
        colsum = np.asarray(Pk[l], np.float32)[:d.N].sum(axis=0)   # [K]
        kvc = nz(np.outer(colsum, ck))                   # [K, DH]
        bul = nz(b2l @ W1l + np.asarray(b1[l], np.float32))
        bol = nz(bo[l])
        bzl = nz(b2[l])
        biases[("cq", l)] = _to_bf16(cq) if cq is not None else None
        biases[("kvc", l)] = kvc.astype(np.float32) if kvc is not None else None
        biases[("kvtc", l)] = kvc.T.astype(np.float32).copy() if kvc is not None else None
        biases[("bu", l)] = _to_bf16(bul) if bul is not None else None
        biases[("bo", l)] = _to_bf16(bol) if bol is not None else None
        biases[("bz", l)] = _to_bf16(bzl) if bzl is not None else None

    lnf_g_rep = np.asarray(lnf_g, np.float32).reshape(1, d.D).copy()
    lnf_b_rep = np.asarray(lnf_b, np.float32).reshape(1, d.D).copy()

    in_maps = []
    for c in range(n_cores):
        m = dict(shared)
        m["x0"] = np.ascontiguousarray(
            x_all[c * d.B_loc:(c + 1) * d.B_loc].reshape(d.R, d.D), dtype=np.float32)
        m["lnfg"] = lnf_g_rep
        m["lnfb"] = lnf_b_rep
        for key, v in biases.items():
            if v is not None:
                m[f"{key[0]}{key[1]}"] = v
        in_maps.append(m)
    return in_maps, biases


def run(dims: Dims, n_cores, inputs, trace=False, tmpdir=None):
    in_maps, biases = prepare_inputs(dims, n_cores, **inputs)
    ck = (dims, n_cores, tuple(sorted(k for k, v in biases.items() if v is not None)))
    if ck not in _CACHE:
        _CACHE[ck] = build(dims, n_cores, biases)
    nc = _CACHE[ck]
    res = run_bass_kernel_spmd(nc, in_maps, list(range(n_cores)), trace=trace,
                               tmpdir=tmpdir)
    outs = np.concatenate([res.results[i]["out"] for i in range(n_cores)], axis=0)
    return outs, res


def kernel(**inputs) -> np.ndarray:
    out, _ = run(Dims(), 8, inputs)
    return out.astype(np.float32)

